# revision 1
# baseline (speedup 1.0000x reference)
"""Trainium2 Bass kernel for nn_AttentionTSSA.

Contract: kernel(**inputs) takes FULL unsharded inputs (numpy), returns the
FULL output. Internally shards batch 16 -> 8 cores x 2 batches, runs one SPMD
Bass program on NeuronCores 0-7 via run_bass_kernel_spmd, and concatenates.

Pipeline (per core, per batch, "transposed" layout [hd, token]):
  x (f32, DRAM) --cast-DMA--> x_bf (bf16, DRAM) --xbar-transpose-DMA--> xT sbuf
  MM1: wT[hd, n] = qkv_wT.T @ xT            (bf16 PE, fp32 psum, ACT evac)
  ww = wT^2, norm2[hd] = sum_n ww           (ACT Square with fused accum)
  sel_rt[hd, h] = temp[head]/max(norm2,eps) (runtime-valued selector)
  s[h, n] = sel_rt.T @ ww                   (PE matmul, col-packed psum)
  E = exp(s)                                (ACT; no max-subtract: s ~ d/N << 1)
  Z[n] = sum_h E (PE ones matmul); Pi12 = E * (1/Z)  (DVE)
  Pi_bc = B.T @ Pi12 (head->hd broadcast)   (PE row-packed matmuls; evac on
      ACT/DVE with fused sigPi accumulation)
  PW = Pi_bc*wT; dots = sum_n PW*wT         (DVE TTs + ACT Copy-accum)
  negattn = -1/(1 + dots/(sigPi+1e-8))
  outT = PW*negattn                         (DVE tensor_scalar, 4x mode)
  MM2: y[tok, j] = outT.T @ out_wT          (bf16 PE; bias folded into the
      DVE psum evacuation as +outb_bc)

Works around a walrus limit (1 sync-wait per instruction) by splitting
multi-wait instructions onto inserted InstNoOp hosts.
"""

import sys

if "/opt/trn_rl_repo" not in sys.path:
    sys.path.insert(0, "/opt/trn_rl_repo")

import numpy as np
import ml_dtypes

import concourse.bass as bass
import concourse.mybir as mybir
import concourse.tile as tile
from concourse.bass_utils import run_bass_kernel_spmd

N_CORES = 8
B, N, DIM, H = 16, 4096, 768, 12
D = DIM // H          # 64
BPC = B // N_CORES    # batches per core = 2
KT = DIM // 128       # 6 k-tiles
CH = 512              # token chunk for matmuls
NCH = N // CH         # 8 chunks
TOKT = N // 128       # 32 token tiles for MM2

F32 = mybir.dt.float32
BF16 = mybir.dt.bfloat16
MULT = mybir.AluOpType.mult
ADD = mybir.AluOpType.add
AF = mybir.ActivationFunctionType

BF_NP = ml_dtypes.bfloat16


def split_multi_waits(nc, max_per_inst=1):
    """Walrus in this container rejects >1 sync wait per instruction; host
    extra waits on InstNoOp instructions inserted just before."""
    ctr = 0
    for f in nc.m.functions:
        for b in f.blocks:
            new_list, changed = [], False
            for i in b.instructions:
                si = i.sync_info
                waits = list(si.on_wait) if si and si.on_wait else []
                if len(waits) > max_per_inst:
                    extras = waits[:-max_per_inst]
                    for w in extras:
                        d = mybir.InstNoOp(name=f"waitsplit-{ctr}", ins=[], outs=[])
                        ctr += 1
                        d.engine = i.engine
                        d.sync_info = mybir.SyncInfo(on_wait=[w], on_update=[])
                        new_list.append(d)
                    si.on_wait = waits[-max_per_inst:]
                    changed = True
                new_list.append(i)
            if changed:
                b.instructions = new_list
    return ctr


def _consts():
    head = np.arange(DIM) // D  # head index per hd column
    selT = np.zeros((128, KT, H), dtype=BF_NP)
    Bsel = np.zeros((H, KT, 128), dtype=BF_NP)
    for c in range(KT):
        for p in range(128):
            h = head[c * 128 + p]
            selT[p, c, h] = 1.0
            Bsel[h, c, p] = 1.0
    Bsel_rep = np.zeros((128, KT, 128), dtype=BF_NP)
    for g in range(4):
        Bsel_rep[32 * g : 32 * g + H] = Bsel
    ones128x1 = np.ones((128, 1), dtype=BF_NP)
    ones128_bf = np.ones((1, 128), dtype=BF_NP)
    ones1x12_f32 = np.ones((1, H), dtype=np.float32)
    return selT, Bsel, Bsel_rep, ones128x1, ones128_bf, ones1x12_f32


HOST_CAST = False


def build_program(split_waits=True, repeat=1, host_cast=None):
    if host_cast is None:
        host_cast = HOST_CAST
    nc = bass.Bass("TRN2", target_bir_lowering=False, debug=False)

    x_d = nc.dram_tensor("x", [BPC, N, DIM], BF16 if host_cast else F32, kind="ExternalInput")
    qkv_d = nc.dram_tensor("qkv_w", [DIM, DIM], F32, kind="ExternalInput")
    temp_d = nc.dram_tensor("temp", [H, 1], F32, kind="ExternalInput")
    ow_d = nc.dram_tensor("out_w", [DIM, DIM], F32, kind="ExternalInput")
    ob_d = nc.dram_tensor("out_b", [DIM], F32, kind="ExternalInput")
    y_d = nc.dram_tensor("out", [BPC, N, DIM], F32, kind="ExternalOutput")

    (selT_np, Bsel_np, Bselrep_np, ones128x1_np, ones128bf_np,
     ones1x12f_np) = _consts()
    Bselrep_c = nc.inline_tensor(Bselrep_np, "Bselrep_c")
    selT_c = nc.inline_tensor(selT_np, "selT_c")
    Bsel_c = nc.inline_tensor(Bsel_np, "Bsel_c")
    ones128x1_c = nc.inline_tensor(ones128x1_np, "ones128x1_c")
    ones128bf_c = nc.inline_tensor(ones128bf_np, "ones128bf_c")
    ones1x12f_c = nc.inline_tensor(ones1x12f_np, "ones1x12f_c")

    with tile.TileContext(nc) as tc:
        with (
            tc.tile_pool(name="consts", bufs=1) as consts,
            tc.tile_pool(name="dram", bufs=1, space="DRAM") as dram,
            tc.tile_pool(name="big", bufs=8) as big,        # xT / ww / Pibc-outT
            tc.tile_pool(name="wt", bufs=7) as wtp,
            tc.tile_pool(name="scr", bufs=3) as scr,
            tc.tile_pool(name="selp", bufs=6) as selp,
            tc.tile_pool(name="ep", bufs=2) as ep,
            tc.tile_pool(name="nvec", bufs=16) as nvec,
            tc.tile_pool(name="rzvec", bufs=2) as rzvec,
            tc.tile_pool(name="stage", bufs=3) as stage,
            tc.tile_pool(name="ps", bufs=4, space="PSUM") as ps,
            tc.tile_pool(name="ps2", bufs=2, space="PSUM") as ps2,
        ):
            # ---- preamble: consts into SBUF ----
            selT = consts.tile([128, KT, H], BF16, tag="selT")
            nc.sync.dma_start(selT[:], selT_c[:])
            Bsel = consts.tile([H, KT, 128], BF16, tag="Bsel")
            nc.sync.dma_start(Bsel[:], Bsel_c[:])
            Bsel_rep = consts.tile([128, KT, 128], BF16, tag="Bsel_rep")
            nc.sync.dma_start(Bsel_rep[:], Bselrep_c[:])
            ones128x1 = consts.tile([128, 1], BF16, tag="ones128x1")
            nc.sync.dma_start(ones128x1[:], ones128x1_c[:])
            ones128bf = consts.tile([1, 128], BF16, tag="ones128bf")
            nc.sync.dma_start(ones128bf[:], ones128bf_c[:])
            ones1x12f = consts.tile([1, H], F32, tag="ones1x12f")
            nc.sync.dma_start(ones1x12f[:], ones1x12f_c[:])

            # qkv: cast to bf16 DRAM, then xbar-transpose into SBUF
            qkv_bf = dram.tile([DIM, DIM], BF16, tag="qkv_bf")
            nc.gpsimd.dma_start(qkv_bf[:], qkv_d[:])
            qkv_wT = consts.tile([128, KT, DIM], BF16, tag="qkv_wT")
            for k in range(KT):
                nc.sync.dma_start(
                    qkv_wT[:, k, :], qkv_bf[:, k * 128 : (k + 1) * 128],
                    transpose=True,
                )

            # x: cast to bf16 DRAM in token chunks; separate DRAM tiles per
            # chunk so each transpose depends only on its own cast. With
            # host_cast the input is already bf16 in DRAM - no bounce.
            TB = 2
            TBS = N // TB
            if not host_cast:
                x_bf = [
                    [
                        dram.tile([TBS, DIM], BF16, tag="x_bf", name=f"xbf{b}_{tb}")
                        for tb in range(TB)
                    ]
                    for b in range(BPC)
                ]
                for b in range(BPC):
                    for tb in range(TB):
                        nc.gpsimd.dma_start(
                            x_bf[b][tb][:], x_d[b, tb * TBS : (tb + 1) * TBS, :]
                        )

            outb_bf = consts.tile([1, DIM], BF16, tag="outb")
            nc.gpsimd.dma_start(
                outb_bf[:], ob_d.ap().rearrange("(a b) -> a b", a=1)
            )
            tempbf = consts.tile([H, 1], BF16, tag="tempbf")
            nc.gpsimd.dma_start(tempbf[:], temp_d[:])
            ow_bf = dram.tile([DIM, DIM], BF16, tag="ow_bf")
            nc.gpsimd.dma_start(ow_bf[:], ow_d[:])
            ow_wT = consts.tile([128, KT, DIM], BF16, tag="ow_wT")
            for k in range(KT):
                nc.sync.dma_start(
                    ow_wT[:, k, :], ow_bf[:, k * 128 : (k + 1) * 128],
                    transpose=True,
                )

            # outb_bc[p, j] = out_b[j] broadcast across partitions (K=1 MM)
            outb_bc = consts.tile([128, DIM], F32, tag="outb_bc")
            for half, (lo, hi) in enumerate([(0, 512), (512, 768)]):
                pst = ps.tile([128, 512], F32, tag="ps", name="obc_ps")
                nc.tensor.matmul(
                    pst[:, 0 : hi - lo], ones128bf[:], outb_bf[:, lo:hi],
                    start=True, stop=True,
                )
                nc.scalar.copy(outb_bc[:, lo:hi], pst[:, 0 : hi - lo])

            # temp_bc[:, c] = temp[head(hd)] for tile c (PE broadcast)
            temp_bc = consts.tile([128, KT], F32, tag="temp_bc")
            for c in range(KT):
                pst = ps.tile([128, 512], F32, tag="ps", name="tmpbc_ps")
                nc.tensor.matmul(
                    pst[:, 0:1], Bsel[:, c, :], tempbf[:], start=True, stop=True
                )
                nc.vector.tensor_copy(temp_bc[:, c : c + 1], pst[:, 0:1])

            # ---- per-batch pipeline (optionally repeated for benching) ----
            def _batch_body():
                for b in range(BPC):
                    # Phase A: xT tiles via xbar transpose (DRAM bf16 -> SBUF),
                    # chunked by token block so they pipeline behind the casts
                    xT = [
                        big.tile([128, N], BF16, tag="big", name=f"xT{b}_{k}")
                        for k in range(KT)
                    ]
                    if host_cast:
                        for tb in range(TB):
                            for k in range(KT):
                                nc.sync.dma_start(
                                    xT[k][:, tb * TBS : (tb + 1) * TBS],
                                    x_d[b, tb * TBS : (tb + 1) * TBS, k * 128 : (k + 1) * 128],
                                    transpose=True,
                                )
                    else:
                        for tb in range(TB):
                            for k in range(KT):
                                nc.sync.dma_start(
                                    xT[k][:, tb * TBS : (tb + 1) * TBS],
                                    x_bf[b][tb][:, k * 128 : (k + 1) * 128],
                                    transpose=True,
                                )

                    # Phase B1: MM1 -> wT tiles (ACT evacuates psum as bf16).
                    # Batch 0 runs token-chunk-outer so matmuls stream behind the
                    # x cast/transpose DMAs; batch 1 runs c-outer to minimize wT
                    # slot pressure while batch 0 is finishing.
                    wT = [
                        wtp.tile([128, N], BF16, tag="wt", name=f"wT{b}_{c}")
                        for c in range(KT)
                    ]

                    def _mm1_chunk(c, j):
                        pst = ps.tile([128, 512], F32, tag="ps", name="mm1ps")
                        for k in range(KT):
                            nc.tensor.matmul(
                                pst[:],
                                qkv_wT[:, k, c * 128 : (c + 1) * 128],
                                xT[k][:, j * CH : (j + 1) * CH],
                                start=(k == 0),
                                stop=(k == KT - 1),
                            )
                        nc.scalar.copy(wT[c][:, j * CH : (j + 1) * CH], pst[:])

                    if b == 0:
                        for j in range(NCH):
                            for c in range(KT):
                                _mm1_chunk(c, j)
                    else:
                        for c in range(KT):
                            for j in range(NCH):
                                _mm1_chunk(c, j)

                    # Phase B2: ww = wT^2 with fused norm2 accum (ACT Square);
                    # scale folded into a runtime-valued selector sel_rt
                    ww = []
                    sel_rts = []
                    for c in range(KT):
                        ww_c = big.tile([128, N], BF16, tag="big", name=f"ww{b}_{c}")
                        norm2 = nvec.tile([128, 8], F32, tag="nvec", name="norm2")
                        nc.scalar.activation(
                            ww_c[:], wT[c][:], AF.Square, accum_out=norm2[:, 0:1]
                        )
                        n2m = nvec.tile([128, 8], F32, tag="nvec", name="n2m")
                        nc.vector.tensor_scalar_max(n2m[:, 0:1], norm2[:, 0:1], 1e-24)
                        rec = nvec.tile([128, 8], F32, tag="nvec", name="rec")
                        nc.vector.reciprocal(rec[:, 0:1], n2m[:, 0:1])
                        scale_c = nvec.tile([128, 8], F32, tag="nvec", name="scale")
                        nc.vector.tensor_tensor(
                            scale_c[:, 0:1], rec[:, 0:1], temp_bc[:, c : c + 1], MULT
                        )
                        sel_rt = selp.tile([128, H], BF16, tag="selrt", name=f"selrt{b}_{c}")
                        nc.vector.memset(sel_rt[:], 0)
                        nc.vector.tensor_copy(
                            sel_rt[0:64, 2 * c : 2 * c + 1], scale_c[0:64, 0:1]
                        )
                        nc.vector.tensor_copy(
                            sel_rt[64:128, 2 * c + 1 : 2 * c + 2], scale_c[64:128, 0:1]
                        )
                        ww.append(ww_c)
                        sel_rts.append(sel_rt)

                    # s[h, n] = sel_rt.T @ ww, col-packed into 2 psum banks
                    s_ps = [ps.tile([128, 512], F32, tag="ps", name=f"s_ps{i}") for i in range(2)]
                    for jp in range(4):
                        for c in range(KT):
                            for half in range(2):
                                j = jp + 4 * half
                                off = 32 * (j % 4)
                                nc.tensor.matmul(
                                    s_ps[half][off : off + H, :],
                                    sel_rts[c][:],
                                    ww[c][:, j * CH : (j + 1) * CH],
                                    start=(c == 0),
                                    stop=(c == KT - 1),
                                    tile_position=(0, off),
                                )

                    # Phase C: E = exp(s); Z = sum_h E; Pi12 = E / Z. Each
                    # chunk j lives at partition offset 32*(j%4) end-to-end, so
                    # the row-packed broadcast matmuls consume Pi12 directly.
                    E_t = ep.tile([128, N], BF16, tag="ep", name="E")
                    for j in range(NCH):
                        off = 32 * (j % 4)
                        nc.scalar.activation(
                            E_t[off : off + H, j * CH : (j + 1) * CH],
                            s_ps[j // 4][off : off + H, :],
                            AF.Exp,
                        )
                    z_ps = [ps.tile([128, 512], F32, tag="ps", name=f"z_ps{i}") for i in range(2)]
                    for j in range(NCH):
                        off = 32 * (j % 4)
                        nc.tensor.matmul(
                            z_ps[j // 4][off : off + 1, :],
                            ones128x1[off : off + H, :],
                            E_t[off : off + H, j * CH : (j + 1) * CH],
                            start=True,
                            stop=True,
                            tile_position=(off, off),
                        )
                    Pi12 = ep.tile([128, N], BF16, tag="ep", name="Pi12")
                    for j in range(NCH):
                        off = 32 * (j % 4)
                        rzc = rzvec.tile([1, CH], F32, tag="recipZ", name="rzc")
                        nc.vector.reciprocal(rzc[:], z_ps[j // 4][off : off + 1, :])
                        pst = ps.tile([128, 512], F32, tag="ps", name="rz12ps")
                        nc.tensor.matmul(
                            pst[off : off + H, :], ones1x12f[:], rzc[:],
                            start=True, stop=True, tile_position=(0, off),
                        )
                        nc.vector.tensor_tensor(
                            Pi12[off : off + H, j * CH : (j + 1) * CH],
                            E_t[off : off + H, j * CH : (j + 1) * CH],
                            pst[off : off + H, :],
                            MULT,
                        )

                    # Phase D: Pi_bc = B.T @ Pi12 (psum); ACT evac with fused
                    # sigPi accumulation (per-hd sum over tokens)
                    Pibc = []
                    sig_bc = []
                    for t in range(KT):
                        Pibc_t = big.tile([128, N], BF16, tag="big", name=f"Pibc{b}_{t}")
                        sig_parts = nvec.tile([128, 8], F32, tag="nvec", name="sigp")
                        for j in range(NCH):
                            g = j % 4
                            pst = ps.tile([128, 512], F32, tag="ps", name="ebcps")
                            nc.tensor.matmul(
                                pst[:],
                                Bsel_rep[32 * g : 32 * g + H, t, :],
                                Pi12[32 * g : 32 * g + H, j * CH : (j + 1) * CH],
                                start=True,
                                stop=True,
                                tile_position=(32 * g, 0),
                            )
                            if j % 2 == 0:
                                nc.scalar.activation(
                                    Pibc_t[:, j * CH : (j + 1) * CH],
                                    pst[:],
                                    AF.Copy,
                                    accum_out=sig_parts[:, j : j + 1],
                                )
                            else:
                                nc.vector.tensor_scalar(
                                    Pibc_t[:, j * CH : (j + 1) * CH],
                                    pst[:],
                                    1.0,
                                    0.0,
                                    MULT,
                                    ADD,
                                    accum_out=sig_parts[:, j : j + 1],
                                )
                        sig_t = nvec.tile([128, 8], F32, tag="nvec", name=f"sig{b}_{t}")
                        nc.vector.tensor_reduce(
                            sig_t[:, 0:1], sig_parts[:], mybir.AxisListType.X, ADD
                        )
                        Pibc.append(Pibc_t)
                        sig_bc.append(sig_t)

                    # Phase F/G: dots, attn, outT (outT goes into the wt pool,
                    # taking the slot of the just-freed wT tile; Pibc slots free
                    # early so the next batch's xT transposes can start)
                    outT = []
                    for t in range(KT):
                        PW_t = scr.tile([128, N], BF16, tag="scr", name=f"PW{b}_{t}")
                        nc.vector.tensor_tensor(PW_t[:], Pibc[t][:], wT[t][:], MULT)
                        t4 = scr.tile([128, N], BF16, tag="scr", name=f"t4{b}_{t}")
                        nc.vector.tensor_tensor(t4[:], PW_t[:], wT[t][:], MULT)
                        dots_pre = nvec.tile([128, 8], F32, tag="nvec", name="dots")
                        djunk = scr.tile([128, N], BF16, tag="scr", name=f"dj{b}_{t}")
                        nc.scalar.activation(
                            djunk[:], t4[:], AF.Copy, accum_out=dots_pre[:, 0:1]
                        )
                        sp = nvec.tile([128, 8], F32, tag="nvec", name="sp")
                        nc.vector.tensor_scalar_add(sp[:, 0:1], sig_bc[t][:, 0:1], 1e-8)
                        rsp = nvec.tile([128, 8], F32, tag="nvec", name="rsp")
                        nc.vector.reciprocal(rsp[:, 0:1], sp[:, 0:1])
                        o1 = nvec.tile([128, 8], F32, tag="nvec", name="o1")
                        nc.vector.tensor_scalar(
                            o1[:, 0:1], dots_pre[:, 0:1], rsp[:, 0:1], 1.0, MULT, ADD
                        )
                        at = nvec.tile([128, 8], F32, tag="nvec", name="at")
                        nc.vector.reciprocal(at[:, 0:1], o1[:, 0:1])
                        negattn = nvec.tile([128, 8], F32, tag="nvec", name="negattn")
                        nc.vector.tensor_scalar_mul(negattn[:, 0:1], at[:, 0:1], -1.0)
                        outT_t = wtp.tile([128, N], BF16, tag="wt", name=f"outT{b}_{t}")
                        nc.vector.tensor_scalar(
                            outT_t[:], PW_t[:], negattn[:, 0:1], 0.0, MULT, ADD
                        )
                        outT.append(outT_t)

                    # Phase H: MM2; bias folded into the DVE evacuation
                    for tc_i in range(TOKT):
                        p2 = ps2.tile([128, DIM], F32, tag="ps2", name="mm2ps")
                        for k in range(KT):
                            lhs = outT[k][:, tc_i * 128 : (tc_i + 1) * 128]
                            nc.tensor.matmul(
                                p2[:, 0:512], lhs, ow_wT[:, k, 0:512],
                                start=(k == 0), stop=(k == KT - 1),
                            )
                            nc.tensor.matmul(
                                p2[:, 512:768], lhs, ow_wT[:, k, 512:768],
                                start=(k == 0), stop=(k == KT - 1),
                            )
                        stg = stage.tile([128, DIM], F32, tag="stage", name="stg")
                        nc.vector.tensor_tensor(stg[:], p2[:], outb_bc[:], ADD)
                        # stores ride the ACT HWDGE ring, keeping the SP
                        # ring free for the xT transposes
                        nc.scalar.dma_start(
                            y_d[b, tc_i * 128 : (tc_i + 1) * 128, :], stg[:]
                        )


            if repeat > 1:
                with tc.For_i(0, repeat, 1, name="bench"):
                    _batch_body()
            else:
                _batch_body()

    if split_waits:
        split_multi_waits(nc)
    nc.finalize()
    return nc


class _Runner:
    """Caches the Bass program, the jitted shard_map callable, and the
    per-core-replicated weights so repeat calls only move x in / out."""

    def __init__(self, nc=None):
        import jax
        from jax.sharding import Mesh, PartitionSpec
        from jax.experimental.shard_map import shard_map
        from concourse import bass2jax
        import concourse.mybir as _mybir

        bass2jax.install_neuronx_cc_hook()
        self.jax = jax
        if nc is None:
            nc = build_program()
        self.nc = nc

        partition_name = (
            nc.partition_id_tensor.name if nc.partition_id_tensor else None
        )
        in_names, out_names, out_avals = [], [], []
        for alloc in nc.m.functions[0].allocations:
            if not isinstance(alloc, _mybir.MemoryLocationSet):
                continue
            name = alloc.memorylocations[0].name
            if alloc.kind == "ExternalInput":
                if name != partition_name:
                    in_names.append(name)
            elif alloc.kind == "ExternalOutput":
                out_names.append(name)
                out_avals.append(
                    jax.core.ShapedArray(
                        tuple(alloc.tensor_shape), _mybir.dt.np(alloc.dtype)
                    )
                )
        self.in_names = list(in_names)
        self.out_names = out_names
        self.out_avals = out_avals
        n_params = len(in_names)
        n_outs = len(out_names)
        all_names = in_names + out_names
        if partition_name is not None:
            all_names = all_names + [partition_name]

        def _body(*args):
            operands = list(args)
            if partition_name is not None:
                operands.append(bass2jax.partition_id_tensor())
            outs = bass2jax._bass_exec_p.bind(
                *operands,
                out_avals=tuple(out_avals),
                in_names=tuple(all_names),
                out_names=tuple(out_names),
                lowering_input_output_aliases=(),
                sim_require_finite=True,
                sim_require_nnan=True,
                nc=nc,
            )
            return tuple(outs)

        devices = jax.devices()[:N_CORES]
        self.mesh = Mesh(np.asarray(devices), ("core",))
        in_specs = (PartitionSpec("core"),) * (n_params + n_outs)
        out_specs = (PartitionSpec("core"),) * n_outs
        self.donate = tuple(range(n_params, n_params + n_outs))
        self.sharded = jax.jit(
            shard_map(
                _body,
                mesh=self.mesh,
                in_specs=in_specs,
                out_specs=out_specs,
                check_rep=False,
            ),
            donate_argnums=self.donate,
            keep_unused=True,
        )
        self.weights_dev = None

    def stage_weights(self, qkv_w, temp, out_w, out_b):
        self.weights_dev = {
            "qkv_w": self.jax.device_put(np.concatenate([qkv_w] * N_CORES, 0)),
            "temp": self.jax.device_put(np.concatenate([temp] * N_CORES, 0)),
            "out_w": self.jax.device_put(np.concatenate([out_w] * N_CORES, 0)),
            "out_b": self.jax.device_put(np.concatenate([out_b] * N_CORES, 0)),
        }

    def zeros_out(self):
        jnp = self.jax.numpy
        return [
            jnp.zeros((N_CORES * a.shape[0],) + a.shape[1:], a.dtype)
            for a in self.out_avals
        ]

    def run_raw(self, x_dev):
        """x_dev: [16, N, DIM] array (host or device). Returns device array."""
        ins = {"x": x_dev, **self.weights_dev}
        args = [ins[n] for n in self.in_names]
        outs = self.sharded(*args, *self.zeros_out())
        return outs[0]

    def __call__(self, x):
        out = self.run_raw(x)
        return np.asarray(out).reshape(B, N, DIM)


_RUNNER = None


def _get_runner():
    global _RUNNER
    if _RUNNER is None:
        _RUNNER = _Runner()
    return _RUNNER


def kernel(x, qkv_w, temp, out_w, out_b):
    x = np.ascontiguousarray(np.asarray(x, dtype=np.float32))
    qkv_w = np.ascontiguousarray(np.asarray(qkv_w, dtype=np.float32))
    temp = np.ascontiguousarray(np.asarray(temp, dtype=np.float32))
    out_w = np.ascontiguousarray(np.asarray(out_w, dtype=np.float32))
    out_b = np.ascontiguousarray(np.asarray(out_b, dtype=np.float32))

    r = _get_runner()
    r.stage_weights(qkv_w, temp, out_w, out_b)
    return r(x)


if __name__ == "__main__":
    rng = np.random.default_rng(0)
    ins = {
        "x": rng.standard_normal((B, N, DIM)).astype(np.float32),
        "qkv_w": (rng.standard_normal((DIM, DIM)) * 0.02).astype(np.float32),
        "temp": np.ones((H, 1), np.float32),
        "out_w": (rng.standard_normal((DIM, DIM)) * 0.02).astype(np.float32),
        "out_b": np.zeros((DIM,), np.float32),
    }
    out = kernel(**ins)
    print("kernel ran, out shape", out.shape, "dtype", out.dtype)

